# revision 1
# baseline (speedup 1.0000x reference)
"""Multi-sense skip-gram (MSSG) loss kernel for Trainium2.

Strategy: data-parallel over the batch across 8 NeuronCores. Embedding
tables are replicated (converted to bf16 on host, which halves gather
traffic; f32 accumulation on-chip keeps the loss accurate).

The three tables are packed row-wise into one [50000, 2100] bf16 table:
row v = [global(300) | emb senses(900) | disamb senses(900)]. One
indirect-DMA gather per context word then fetches the context embedding
AND both sense blocks in a single 4200B contiguous descriptor per
partition. (The HW SWDGE indirect-gather ucode consumes exactly one
index per destination partition-row, so each gather call uses a [P,1]
index column — verified against hardware.)

Each core processes 512 batch elements as 4 tiles of 128 (one element
per SBUF partition). DVE does the batched dot products / weighted sums
along the free dimension; ACT does exp/ln; the final scalar partial is
reduced across partitions with a ones-vector matmul on the PE. Host
sums the 8 per-core partials.
"""

import numpy as np

NUM_SENSE = 3
EMB_DIM = 300
VOCAB = 50000
BATCH = 4096
CTX = 10
NEG = 5
N_CORES = 8
P = 128
PER_CORE = BATCH // N_CORES  # 512
TILES = PER_CORE // P        # 4
D = EMB_DIM
CS = CTX * NUM_SENSE         # 30
SN = NUM_SENSE * NEG         # 15
RowLen = D + 2 * NUM_SENSE * D  # 2100: [glob | emb | dis]
EMB_OFF = D                  # 300
DIS_OFF = D + NUM_SENSE * D  # 1200

USE_STT_DOT = False  # fused multiply+accum_out dot products
USE_STT_ACC = False  # fused FMA chain for weighted sums

_CACHE = {}


def _build_bass(tiles=TILES, passes=1):
    key = ("nc", USE_STT_DOT, USE_STT_ACC, tiles, passes)
    if key in _CACHE:
        return _CACHE[key]

    import concourse.bass as bass
    import concourse.bacc as bacc
    import concourse.tile as tile
    from concourse import mybir

    F32 = mybir.dt.float32
    BF16 = mybir.dt.bfloat16
    I32 = mybir.dt.int32
    AX = mybir.AxisListType
    OP = mybir.AluOpType
    AF = mybir.ActivationFunctionType
    TINY = float(np.finfo(np.float32).tiny)

    nc = bacc.Bacc("TRN2", target_bir_lowering=False, debug=False)

    packed = nc.dram_tensor("packed", [VOCAB, RowLen], BF16, kind="ExternalInput")
    idx = nc.dram_tensor("idx", [PER_CORE, 16], I32, kind="ExternalInput")
    out_d = nc.dram_tensor("out", [1, 1], F32, kind="ExternalOutput")

    def tt(out, a, b, op=OP.add):
        nc.vector.tensor_tensor(out=out, in0=a, in1=b, op=op)

    with tile.TileContext(nc) as tc:
        with (
            tc.tile_pool(name="gather", bufs=2) as gp,
            tc.tile_pool(name="tmpp", bufs=2) as tp,
            tc.tile_pool(name="small", bufs=2) as sp,
            tc.tile_pool(name="persist", bufs=1) as pp,
            tc.tile_pool(name="psum", bufs=1, space="PSUM") as psp,
        ):
            acc = pp.tile([P, 2 * TILES], F32)
            ones = pp.tile([P, 1], F32)
            nc.vector.memset(ones[:], 1.0)

            for t_iter in range(tiles * passes):
                t = t_iter % tiles
                rows = slice(t * P, (t + 1) * P)
                ix = gp.tile([P, 16], I32)
                nc.sync.dma_start(out=ix[:], in_=idx[rows, :])

                # one gather call per index column (HW: 1 idx / partition-row)
                PK = gp.tile([P, CTX * RowLen], BF16)   # ctx: [glob|emb|dis] x 10
                NG = gp.tile([P, NEG * D], BF16)        # neg: glob rows
                WD = gp.tile([P, RowLen], BF16)         # word: full row

                def gather1(dst, offs):
                    nc.gpsimd.indirect_dma_start(
                        out=dst, out_offset=None, in_=packed[:],
                        in_offset=bass.IndirectOffsetOnAxis(ap=offs, axis=0),
                    )

                for k in range(CTX):
                    gather1(PK[:, k * RowLen:(k + 1) * RowLen], ix[:, k:k + 1])
                for k in range(NEG):
                    # out row shorter than table row -> fetches first 300
                    # elements of the row = the global embedding
                    gather1(NG[:, k * D:(k + 1) * D], ix[:, CTX + k:CTX + k + 1])
                gather1(WD[:], ix[:, 15:16])

                PK4 = PK[:].rearrange("p (c x) -> p c x", x=RowLen)
                CT3 = PK4[:, :, 0:D]                                   # [P,10,300]
                AS4 = PK4[:, :, EMB_OFF:DIS_OFF].rearrange(
                    "p c (s d) -> p c s d", d=D)                       # [P,10,3,300]
                AD4 = PK4[:, :, DIS_OFF:RowLen].rearrange(
                    "p c (s d) -> p c s d", d=D)
                SEN3 = WD[:, EMB_OFF:DIS_OFF].rearrange(
                    "p (s d) -> p s d", d=D)                           # [P,3,300]
                DIS3 = WD[:, DIS_OFF:RowLen].rearrange(
                    "p (s d) -> p s d", d=D)
                NG3 = NG[:].rearrange("p (n d) -> p n d", d=D)

                def as_seg(k):
                    c, s = k // NUM_SENSE, k % NUM_SENSE
                    return AS4[:, c, s, :]

                def ad_seg(k):
                    c, s = k // NUM_SENSE, k % NUM_SENSE
                    return AD4[:, c, s, :]

                TMP = tp.tile([P, CS * D], BF16)

                def seg3(ap):
                    return ap.rearrange("p (c d) -> p c d", d=D)

                def dot_segments(a_seg, b_seg, a_full, b_full, out_full,
                                 zout, nseg):
                    """zout[:, k] = sum_d a_seg(k) * b_seg(k)  (dots of 300-vecs).

                    a_seg/b_seg: k -> [P,300] AP (fused STT path).
                    a_full/b_full/out_full: shape-matched APs covering all
                    nseg segments (mult+fold-tree path); out_full views TMP.
                    """
                    if USE_STT_DOT:
                        T3 = seg3(TMP[:])
                        for k in range(nseg):
                            nc.vector.scalar_tensor_tensor(
                                out=T3[:, k, :], in0=a_seg(k), scalar=1.0,
                                in1=b_seg(k),
                                op0=OP.mult, op1=OP.mult,
                                accum_out=zout[:, k:k + 1])
                    else:
                        tt(out_full, a_full, b_full, OP.mult)
                        pr = seg3(TMP[:])[:, 0:nseg, :]
                        tt(pr[:, :, 0:150], pr[:, :, 0:150], pr[:, :, 150:300])
                        tt(pr[:, :, 0:74], pr[:, :, 0:74], pr[:, :, 76:150])
                        tt(pr[:, :, 0:38], pr[:, :, 0:38], pr[:, :, 38:76])
                        nc.vector.tensor_reduce(
                            out=zout, in_=pr[:, :, 0:38], axis=AX.X, op=OP.add)

                # ---- ctx1 = sum_c CT (mean deferred via exp-scale) ----
                c1a = sp.tile([P, 5 * D], BF16)
                c1b = sp.tile([P, 2 * D], BF16)
                c1c = sp.tile([P, D], BF16)
                ctx1 = sp.tile([P, D], BF16)
                tt(seg3(c1a[:]), CT3[:, 0:5, :], CT3[:, 5:10, :])
                tt(c1b[:], c1a[:, 0:2 * D], c1a[:, 2 * D:4 * D])
                tt(c1c[:], c1b[:, 0:D], c1b[:, D:2 * D])
                tt(ctx1[:], c1c[:], c1a[:, 4 * D:5 * D])

                def disamb_step(ctx_vec, ctx_out):
                    z = sp.tile([P, CS], F32, tag="z")
                    dot_segments(
                        ad_seg, lambda k: ctx_vec[:],
                        AD4,
                        ctx_vec[:].unsqueeze(1).unsqueeze(1)
                                  .to_broadcast([P, CTX, NUM_SENSE, D]),
                        TMP[:].rearrange("p (c s d) -> p c s d",
                                         s=NUM_SENSE, d=D),
                        z[:], CS)
                    E = sp.tile([P, CS], F32, tag="E")
                    nc.scalar.activation(out=E[:], in_=z[:], func=AF.Exp,
                                         scale=1.0 / CTX)
                    S = sp.tile([P, CTX], F32, tag="S")
                    nc.vector.tensor_reduce(
                        out=S[:], in_=E[:].rearrange("p (c s) -> p c s",
                                                     s=NUM_SENSE),
                        axis=AX.X, op=OP.add)
                    R = sp.tile([P, CTX], F32, tag="R")
                    nc.vector.reciprocal(R[:], S[:])
                    AL = sp.tile([P, CS], F32, tag="AL")
                    tt(AL[:].rearrange("p (c s) -> p c s", s=NUM_SENSE),
                       E[:].rearrange("p (c s) -> p c s", s=NUM_SENSE),
                       R[:].unsqueeze(2).to_broadcast([P, CTX, NUM_SENSE]),
                       OP.mult)
                    if USE_STT_ACC:
                        a0 = sp.tile([P, D], BF16, tag="acc0")
                        nc.vector.tensor_scalar_mul(
                            out=a0[:], in0=as_seg(0), scalar1=AL[:, 0:1])
                        for cs in range(1, CS - 1):
                            nc.vector.scalar_tensor_tensor(
                                out=a0[:], in0=as_seg(cs),
                                scalar=AL[:, cs:cs + 1], in1=a0[:],
                                op0=OP.mult, op1=OP.add)
                        nc.vector.scalar_tensor_tensor(
                            out=ctx_out[:], in0=as_seg(CS - 1),
                            scalar=AL[:, CS - 1:CS], in1=a0[:],
                            op0=OP.mult, op1=OP.add)
                    else:
                        T3 = seg3(TMP[:])
                        for cs in range(CS):
                            nc.vector.tensor_scalar_mul(
                                out=T3[:, cs, :], in0=as_seg(cs),
                                scalar1=AL[:, cs:cs + 1])
                        tt(T3[:, 0:15, :], T3[:, 0:15, :], T3[:, 15:30, :])
                        tt(T3[:, 0:7, :], T3[:, 0:7, :], T3[:, 7:14, :])
                        tt(T3[:, 0:3, :], T3[:, 0:3, :], T3[:, 3:6, :])
                        tt(T3[:, 0:1, :], T3[:, 0:1, :], T3[:, 1:2, :])
                        tt(T3[:, 0:1, :], T3[:, 0:1, :], T3[:, 14:15, :])
                        tt(T3[:, 2:3, :], T3[:, 2:3, :], T3[:, 6:7, :])
                        tt(ctx_out[:].unsqueeze(1), T3[:, 0:1, :], T3[:, 2:3, :])

                ctx2 = sp.tile([P, D], BF16)
                ctx3 = sp.tile([P, D], BF16)
                disamb_step(ctx1, ctx2)
                disamb_step(ctx2, ctx3)

                # ---- alpha = softmax_s(DIS . ctx3/C) ----
                av = sp.tile([P, NUM_SENSE], F32)
                dot_segments(
                    lambda k: DIS3[:, k, :], lambda k: ctx3[:],
                    DIS3,
                    ctx3[:].unsqueeze(1).to_broadcast([P, NUM_SENSE, D]),
                    seg3(TMP[:])[:, 0:NUM_SENSE, :], av[:], NUM_SENSE)
                Ea = sp.tile([P, NUM_SENSE], F32)
                nc.scalar.activation(out=Ea[:], in_=av[:], func=AF.Exp,
                                     scale=1.0 / CTX)
                Sa = sp.tile([P, 1], F32)
                nc.vector.tensor_reduce(out=Sa[:], in_=Ea[:], axis=AX.X, op=OP.add)
                Ra = sp.tile([P, 1], F32)
                nc.vector.reciprocal(Ra[:], Sa[:])
                ALS = sp.tile([P, NUM_SENSE], F32)
                nc.vector.tensor_scalar_mul(out=ALS[:], in0=Ea[:], scalar1=Ra[:, 0:1])

                # ---- pos: sigmoid(SEN . CT) over (s,c) ----
                PL = sp.tile([P, CS], F32)
                dot_segments(
                    lambda k: CT3[:, k % CTX, :], lambda k: SEN3[:, k // CTX, :],
                    CT3.unsqueeze(1).to_broadcast([P, NUM_SENSE, CTX, D]),
                    SEN3.unsqueeze(2).to_broadcast([P, NUM_SENSE, CTX, D]),
                    TMP[:].rearrange("p (s c d) -> p s c d", s=NUM_SENSE, c=CTX),
                    PL[:], CS)
                EP = sp.tile([P, CS], F32)
                nc.scalar.activation(out=EP[:], in_=PL[:], func=AF.Exp, scale=-1.0)
                nc.vector.tensor_scalar_add(out=EP[:], in0=EP[:], scalar1=1.0)
                PP = sp.tile([P, CS], F32)
                nc.vector.reciprocal(PP[:], EP[:])

                # pos_term[c] = sum_s alpha_s * PP[s,c]
                W = sp.tile([P, CS], F32)
                for s in range(NUM_SENSE):
                    nc.vector.tensor_scalar_mul(
                        out=W[:, s * CTX:(s + 1) * CTX],
                        in0=PP[:, s * CTX:(s + 1) * CTX], scalar1=ALS[:, s:s + 1])
                tt(W[:, 0:CTX], W[:, 0:CTX], W[:, CTX:2 * CTX])
                tt(W[:, 0:CTX], W[:, 0:CTX], W[:, 2 * CTX:3 * CTX])
                nc.vector.tensor_scalar_max(out=W[:, 0:CTX], in0=W[:, 0:CTX],
                                            scalar1=TINY)
                WL = sp.tile([P, CTX], F32)
                nc.scalar.activation(out=WL[:], in_=W[:, 0:CTX], func=AF.Ln,
                                     accum_out=acc[:, 2 * t:2 * t + 1])

                # ---- neg: sigmoid(SEN . NG) over (s,n) ----
                NL = sp.tile([P, SN], F32)
                dot_segments(
                    lambda k: NG3[:, k % NEG, :], lambda k: SEN3[:, k // NEG, :],
                    NG3.unsqueeze(1).to_broadcast([P, NUM_SENSE, NEG, D]),
                    SEN3.unsqueeze(2).to_broadcast([P, NUM_SENSE, NEG, D]),
                    TMP[:, 0:SN * D].rearrange("p (s n d) -> p s n d",
                                               s=NUM_SENSE, n=NEG),
                    NL[:], SN)
                EN = sp.tile([P, SN], F32)
                nc.scalar.activation(out=EN[:], in_=NL[:], func=AF.Exp, scale=-1.0)
                nc.vector.tensor_scalar_add(out=EN[:], in0=EN[:], scalar1=1.0)
                NP = sp.tile([P, SN], F32)
                nc.vector.reciprocal(NP[:], EN[:])

                Wn = sp.tile([P, SN], F32)
                for s in range(NUM_SENSE):
                    nc.vector.tensor_scalar_mul(
                        out=Wn[:, s * NEG:(s + 1) * NEG],
                        in0=NP[:, s * NEG:(s + 1) * NEG], scalar1=ALS[:, s:s + 1])
                tt(Wn[:, 0:NEG], Wn[:, 0:NEG], Wn[:, NEG:2 * NEG])
                tt(Wn[:, 0:NEG], Wn[:, 0:NEG], Wn[:, 2 * NEG:3 * NEG])
                # 1 - x, clamp, ln
                nc.vector.tensor_scalar(
                    out=Wn[:, NEG:2 * NEG], in0=Wn[:, 0:NEG],
                    scalar1=-1.0, scalar2=1.0, op0=OP.mult, op1=OP.add)
                nc.vector.tensor_scalar_max(
                    out=Wn[:, NEG:2 * NEG], in0=Wn[:, NEG:2 * NEG], scalar1=TINY)
                WLn = sp.tile([P, NEG], F32)
                nc.scalar.activation(out=WLn[:], in_=Wn[:, NEG:2 * NEG], func=AF.Ln,
                                     accum_out=acc[:, 2 * t + 1:2 * t + 2])

            total = pp.tile([P, 1], F32)
            nc.vector.tensor_reduce(out=total[:], in_=acc[:], axis=AX.X, op=OP.add)
            ps = psp.tile([1, 1], F32)
            nc.tensor.matmul(out=ps[:], lhsT=total[:], rhs=ones[:],
                             start=True, stop=True)
            res = pp.tile([1, 1], F32)
            nc.scalar.copy(res[:], ps[:])
            nc.sync.dma_start(out=out_d[:, :], in_=res[:])

    nc.compile()
    _CACHE[key] = nc
    return nc


def _prep_inputs(word_ids, context_ids, neg_ids,
                 emb_weight, global_emb_weight, disamb_weight):
    import ml_dtypes
    BF = ml_dtypes.bfloat16
    word_ids = np.asarray(word_ids).astype(np.int32).reshape(BATCH)
    context_ids = np.asarray(context_ids).astype(np.int32).reshape(BATCH, CTX)
    neg_ids = np.asarray(neg_ids).astype(np.int32).reshape(BATCH, NEG)
    packed = np.empty((VOCAB, RowLen), dtype=BF)
    packed[:, 0:D] = np.asarray(
        global_emb_weight, dtype=np.float32).astype(BF).reshape(VOCAB, D)
    packed[:, EMB_OFF:DIS_OFF] = np.asarray(
        emb_weight, dtype=np.float32).astype(BF).reshape(VOCAB, NUM_SENSE * D)
    packed[:, DIS_OFF:RowLen] = np.asarray(
        disamb_weight, dtype=np.float32).astype(BF).reshape(VOCAB, NUM_SENSE * D)
    idx_all = np.concatenate(
        [context_ids, neg_ids, word_ids[:, None]], axis=1).astype(np.int32)
    in_maps = []
    for c in range(N_CORES):
        sl = slice(c * PER_CORE, (c + 1) * PER_CORE)
        in_maps.append({
            "packed": packed,
            "idx": np.ascontiguousarray(idx_all[sl]),
        })
    return in_maps


def kernel(word_ids, context_ids, context_masks, neg_ids,
           emb_weight, global_emb_weight, disamb_weight):
    from concourse import bass_utils
    nc = _build_bass()
    in_maps = _prep_inputs(word_ids, context_ids, neg_ids,
                           emb_weight, global_emb_weight, disamb_weight)
    res = bass_utils.run_bass_kernel_spmd(nc, in_maps, core_ids=list(range(N_CORES)))
    total = 0.0
    for r in res.results:
        total += float(np.asarray(r["out"]).reshape(-1)[0])
    loss = -total / float(BATCH * CTX)
    return np.array(loss, dtype=np.float32)



# revision 3
# speedup vs baseline: 133.4684x; 133.4684x over previous
"""Multi-sense skip-gram (MSSG) loss kernel for Trainium2.

Data-parallel over batch across 8 cores; tables packed row-wise into one
[50000, 2100] bf16 table: row v = [global(300) | emb senses(900) | disamb
senses(900)].

Structure (per 128-element tile, 4 tiles per core):
- 16 single-index indirect gathers (one per ctx/word/neg id column) with
  flat 2D SBUF destinations. NOTE: multi-index gathers ([P,K] offset APs)
  and 3D unit-dim dest views pass CoreSim but CRASH or corrupt on real
  trn2 hardware - do not "merge" these calls.
- Dot products as big broadcast tensor_tensor multiplies (DVE 2x mode)
  + lopsided fold trees (150/74/38) + narrow 1x tensor_reduce.
- Weighted sums via 30x tensor_scalar_mul + fold tree on DVE.
- pos/neg dot reductions offloaded to the Scalar (ACT) engine as
  per-segment Copy+accum_out ops; sigmoid/exp/ln also on ACT. This
  rebalances the baseline (84% DVE busy -> DVE 227us/ACT 145us of a
  ~285us span).
- SBUF->SBUF dma_start(accum_op=add) folds also crash real HW; keep
  folds on DVE.
"""

import numpy as np

NUM_SENSE = 3
EMB_DIM = 300
VOCAB = 50000
BATCH = 4096
CTX = 10
NEG = 5
N_CORES = 8
P = 128
PER_CORE = BATCH // N_CORES  # 512
TILES = PER_CORE // P        # 4
D = EMB_DIM
CS = CTX * NUM_SENSE         # 30
SN = NUM_SENSE * NEG         # 15
RowLen = D + 2 * NUM_SENSE * D  # 2100: [glob | emb | dis]
EMB_OFF = D                  # 300
DIS_OFF = D + NUM_SENSE * D  # 1200

_CACHE = {}


def _build_bass():
    key = "nc"
    if key in _CACHE:
        return _CACHE[key]

    import concourse.bass as bass
    import concourse.bacc as bacc
    import concourse.tile as tile
    from concourse import mybir

    F32 = mybir.dt.float32
    BF16 = mybir.dt.bfloat16
    I32 = mybir.dt.int32
    AX = mybir.AxisListType
    OP = mybir.AluOpType
    AF = mybir.ActivationFunctionType
    TINY = float(np.finfo(np.float32).tiny)

    nc = bacc.Bacc("TRN2", target_bir_lowering=False, debug=False)

    packed = nc.dram_tensor("packed", [VOCAB, RowLen], BF16, kind="ExternalInput")
    # idx columns: 0..9 ctx, 10 word, 11..15 neg
    idx = nc.dram_tensor("idx", [PER_CORE, 16], I32, kind="ExternalInput")
    out_d = nc.dram_tensor("out", [1, 1], F32, kind="ExternalOutput")

    def tt(out, a, b, op=OP.add):
        nc.vector.tensor_tensor(out=out, in0=a, in1=b, op=op)

    with tile.TileContext(nc) as tc:
        with (
            tc.tile_pool(name="gather", bufs=2) as gp,
            tc.tile_pool(name="tmpp", bufs=2) as tp,
            tc.tile_pool(name="small", bufs=2) as sp,
            tc.tile_pool(name="persist", bufs=1) as pp,
            tc.tile_pool(name="psum", bufs=1, space="PSUM") as psp,
        ):
            acc = pp.tile([P, 2 * TILES], F32)
            ones = pp.tile([P, 1], F32)
            nc.vector.memset(ones[:], 1.0)

            for t in range(TILES):
                rows = slice(t * P, (t + 1) * P)
                ix = gp.tile([P, 16], I32)
                nc.sync.dma_start(out=ix[:], in_=idx[rows, :])

                PK = gp.tile([P, 11 * RowLen], BF16)   # 10 ctx + word, full rows
                NG = gp.tile([P, NEG * D], BF16)       # neg: glob part only
                PK3 = PK[:].rearrange("p (k x) -> p k x", x=RowLen)
                NG3 = NG[:].rearrange("p (n d) -> p n d", d=D)

                def gather1(dst, offs):
                    nc.gpsimd.indirect_dma_start(
                        out=dst, out_offset=None, in_=packed[:],
                        in_offset=bass.IndirectOffsetOnAxis(ap=offs, axis=0),
                    )

                for k in range(11):
                    gather1(PK[:, k * RowLen:(k + 1) * RowLen], ix[:, k:k + 1])
                for k in range(NEG):
                    gather1(NG[:, k * D:(k + 1) * D], ix[:, 11 + k:12 + k])

                CT3 = PK3[:, 0:CTX, 0:D]                               # [P,10,300]
                AS4 = PK3[:, 0:CTX, EMB_OFF:DIS_OFF].rearrange(
                    "p c (s d) -> p c s d", d=D)                       # [P,10,3,300]
                AD4 = PK3[:, 0:CTX, DIS_OFF:RowLen].rearrange(
                    "p c (s d) -> p c s d", d=D)
                SEN3 = PK3[:, CTX, EMB_OFF:DIS_OFF].rearrange(
                    "p (s d) -> p s d", d=D)                           # [P,3,300]
                DIS3 = PK3[:, CTX, DIS_OFF:RowLen].rearrange(
                    "p (s d) -> p s d", d=D)

                TMP = tp.tile([P, CS * D], BF16, tag="tmp_d")
                T3 = TMP[:].rearrange("p (k d) -> p k d", d=D)
                TMPQ = tp.tile([P, CS * D], BF16, tag="tmp_pn")
                Q3 = TMPQ[:].rearrange("p (k d) -> p k d", d=D)
                SCR = [sp.tile([P, D], BF16, tag=f"scr{i}", name=f"scr{i}")
                       for i in range(4)]

                def fold_reduce_30(src3, zout):
                    """src3 [P,30,300] bf16 products -> zout [P,30] f32 sums."""
                    tt(src3[:, :, 0:150], src3[:, :, 0:150], src3[:, :, 150:300])
                    tt(src3[:, :, 0:74], src3[:, :, 0:74], src3[:, :, 76:150])
                    tt(src3[:, :, 0:38], src3[:, :, 0:38], src3[:, :, 38:76])
                    nc.vector.tensor_reduce(
                        out=zout, in_=src3[:, :, 0:38], axis=AX.X, op=OP.add)

                def act_reduce(src3, zout, nseg):
                    """per-segment [P,300] Copy+accum on ACT -> zout[:, k]."""
                    for k in range(nseg):
                        nc.scalar.activation(
                            out=SCR[k % 4][:], in_=src3[:, k, :], func=AF.Copy,
                            accum_out=zout[:, k:k + 1])

                # ---- ctx1 = sum_c CT (mean deferred via exp scale) ----
                c1a = sp.tile([P, 5 * D], BF16)
                c1b = sp.tile([P, 2 * D], BF16)
                ctx1 = sp.tile([P, D], BF16)
                c1a3 = c1a[:].rearrange("p (c d) -> p c d", d=D)
                tt(c1a3, CT3[:, 0:5, :], CT3[:, 5:10, :])
                tt(c1b[:], c1a[:, 0:2 * D], c1a[:, 2 * D:4 * D])
                tt(c1b[:, 0:D], c1b[:, 0:D], c1b[:, D:2 * D])
                tt(ctx1[:], c1b[:, 0:D], c1a[:, 4 * D:5 * D])

                def disamb_step(ctx_vec, ctx_out, si):
                    # products + fold + reduce (DVE)
                    tt(T3.rearrange("p (c s) d -> p c s d", s=NUM_SENSE),
                       AD4,
                       ctx_vec[:].unsqueeze(1).unsqueeze(1)
                               .to_broadcast([P, CTX, NUM_SENSE, D]),
                       OP.mult)
                    z = sp.tile([P, CS], F32, tag=f"z{si}")
                    fold_reduce_30(T3, z[:])
                    # softmax over s (ACT exp, DVE small ops)
                    E = sp.tile([P, CS], F32, tag=f"E{si}")
                    nc.scalar.activation(out=E[:], in_=z[:], func=AF.Exp,
                                         scale=1.0 / CTX)
                    S = sp.tile([P, CTX], F32, tag=f"S{si}")
                    nc.vector.tensor_reduce(
                        out=S[:], in_=E[:].rearrange("p (c s) -> p c s",
                                                     s=NUM_SENSE),
                        axis=AX.X, op=OP.add)
                    R = sp.tile([P, CTX], F32, tag=f"R{si}")
                    nc.vector.reciprocal(R[:], S[:])
                    AL = sp.tile([P, CS], F32, tag=f"AL{si}")
                    tt(AL[:].rearrange("p (c s) -> p c s", s=NUM_SENSE),
                       E[:].rearrange("p (c s) -> p c s", s=NUM_SENSE),
                       R[:].unsqueeze(2).to_broadcast([P, CTX, NUM_SENSE]),
                       OP.mult)
                    # weighted sum: 30x tensor_scalar (4x mode) + fold tree
                    for k in range(CS):
                        nc.vector.tensor_scalar_mul(
                            out=T3[:, k, :],
                            in0=AS4[:, k // NUM_SENSE, k % NUM_SENSE, :],
                            scalar1=AL[:, k:k + 1])
                    tt(T3[:, 0:15, :], T3[:, 0:15, :], T3[:, 15:30, :])
                    tt(T3[:, 0:7, :], T3[:, 0:7, :], T3[:, 7:14, :])
                    tt(T3[:, 0:3, :], T3[:, 0:3, :], T3[:, 3:6, :])
                    tt(T3[:, 0:1, :], T3[:, 0:1, :], T3[:, 1:2, :])
                    tt(T3[:, 0:1, :], T3[:, 0:1, :], T3[:, 14:15, :])
                    tt(T3[:, 2:3, :], T3[:, 2:3, :], T3[:, 6:7, :])
                    tt(ctx_out[:].unsqueeze(1), T3[:, 0:1, :], T3[:, 2:3, :])

                ctx2 = sp.tile([P, D], BF16)
                ctx3 = sp.tile([P, D], BF16)
                disamb_step(ctx1, ctx2, 0)
                disamb_step(ctx2, ctx3, 1)

                # ---- alpha = softmax_s(DIS . ctx3 / CTX) ----
                tt(T3[:, 0:NUM_SENSE, :], DIS3,
                   ctx3[:].unsqueeze(1).to_broadcast([P, NUM_SENSE, D]),
                   OP.mult)
                av = sp.tile([P, NUM_SENSE], F32)
                a3 = T3[:, 0:NUM_SENSE, :]
                tt(a3[:, :, 0:150], a3[:, :, 0:150], a3[:, :, 150:300])
                tt(a3[:, :, 0:74], a3[:, :, 0:74], a3[:, :, 76:150])
                tt(a3[:, :, 0:38], a3[:, :, 0:38], a3[:, :, 38:76])
                nc.vector.tensor_reduce(
                    out=av[:], in_=a3[:, :, 0:38], axis=AX.X, op=OP.add)
                Ea = sp.tile([P, NUM_SENSE], F32)
                Sa = sp.tile([P, 1], F32)
                nc.scalar.activation(out=Ea[:], in_=av[:], func=AF.Exp,
                                     scale=1.0 / CTX, accum_out=Sa[:, 0:1])
                Ra = sp.tile([P, 1], F32)
                nc.vector.reciprocal(Ra[:], Sa[:])
                ALS = sp.tile([P, NUM_SENSE], F32)
                nc.vector.tensor_scalar_mul(out=ALS[:], in0=Ea[:],
                                            scalar1=Ra[:, 0:1])

                # ---- pos: sigmoid(SEN . CT) over (s,c) ----
                tt(Q3.rearrange("p (s c) d -> p s c d", c=CTX),
                   CT3.unsqueeze(1).to_broadcast([P, NUM_SENSE, CTX, D]),
                   SEN3.unsqueeze(2).to_broadcast([P, NUM_SENSE, CTX, D]),
                   OP.mult)
                PL = sp.tile([P, CS], F32)
                act_reduce(Q3, PL[:], CS)
                PP = sp.tile([P, CS], F32)
                nc.scalar.activation(out=PP[:], in_=PL[:], func=AF.Sigmoid)

                W = sp.tile([P, CS], F32)
                for s in range(NUM_SENSE):
                    nc.vector.tensor_scalar_mul(
                        out=W[:, s * CTX:(s + 1) * CTX],
                        in0=PP[:, s * CTX:(s + 1) * CTX],
                        scalar1=ALS[:, s:s + 1])
                tt(W[:, 0:CTX], W[:, 0:CTX], W[:, CTX:2 * CTX])
                tt(W[:, 0:CTX], W[:, 0:CTX], W[:, 2 * CTX:3 * CTX])
                nc.vector.tensor_scalar_max(out=W[:, 0:CTX], in0=W[:, 0:CTX],
                                            scalar1=TINY)
                WL = sp.tile([P, CTX], F32)
                nc.scalar.activation(out=WL[:], in_=W[:, 0:CTX], func=AF.Ln,
                                     accum_out=acc[:, 2 * t:2 * t + 1])

                # ---- neg: sigmoid(SEN . NG) over (s,n) ----
                tt(Q3[:, 0:SN, :].rearrange("p (s n) d -> p s n d", n=NEG),
                   NG3.unsqueeze(1).to_broadcast([P, NUM_SENSE, NEG, D]),
                   SEN3.unsqueeze(2).to_broadcast([P, NUM_SENSE, NEG, D]),
                   OP.mult)
                NL = sp.tile([P, SN], F32)
                act_reduce(Q3, NL[:], SN)
                NP = sp.tile([P, SN], F32)
                nc.scalar.activation(out=NP[:], in_=NL[:], func=AF.Sigmoid)

                Wn = sp.tile([P, SN], F32)
                for s in range(NUM_SENSE):
                    nc.vector.tensor_scalar_mul(
                        out=Wn[:, s * NEG:(s + 1) * NEG],
                        in0=NP[:, s * NEG:(s + 1) * NEG],
                        scalar1=ALS[:, s:s + 1])
                tt(Wn[:, 0:NEG], Wn[:, 0:NEG], Wn[:, NEG:2 * NEG])
                tt(Wn[:, 0:NEG], Wn[:, 0:NEG], Wn[:, 2 * NEG:3 * NEG])
                # 1 - x, clamp, ln
                nc.vector.tensor_scalar(
                    out=Wn[:, NEG:2 * NEG], in0=Wn[:, 0:NEG],
                    scalar1=-1.0, scalar2=1.0, op0=OP.mult, op1=OP.add)
                nc.vector.tensor_scalar_max(
                    out=Wn[:, NEG:2 * NEG], in0=Wn[:, NEG:2 * NEG], scalar1=TINY)
                WLn = sp.tile([P, NEG], F32)
                nc.scalar.activation(out=WLn[:], in_=Wn[:, NEG:2 * NEG],
                                     func=AF.Ln,
                                     accum_out=acc[:, 2 * t + 1:2 * t + 2])

            total = pp.tile([P, 1], F32)
            nc.vector.tensor_reduce(out=total[:], in_=acc[:], axis=AX.X, op=OP.add)
            ps = psp.tile([1, 1], F32)
            nc.tensor.matmul(out=ps[:], lhsT=total[:], rhs=ones[:],
                             start=True, stop=True)
            res = pp.tile([1, 1], F32)
            nc.scalar.copy(res[:], ps[:])
            nc.sync.dma_start(out=out_d[:, :], in_=res[:])

    nc.compile()
    _CACHE[key] = nc
    return nc


def _prep_inputs(word_ids, context_ids, neg_ids,
                 emb_weight, global_emb_weight, disamb_weight):
    import ml_dtypes
    BF = ml_dtypes.bfloat16
    word_ids = np.asarray(word_ids).astype(np.int32).reshape(BATCH)
    context_ids = np.asarray(context_ids).astype(np.int32).reshape(BATCH, CTX)
    neg_ids = np.asarray(neg_ids).astype(np.int32).reshape(BATCH, NEG)
    packed = np.empty((VOCAB, RowLen), dtype=BF)
    packed[:, 0:D] = np.asarray(
        global_emb_weight, dtype=np.float32).astype(BF).reshape(VOCAB, D)
    packed[:, EMB_OFF:DIS_OFF] = np.asarray(
        emb_weight, dtype=np.float32).astype(BF).reshape(VOCAB, NUM_SENSE * D)
    packed[:, DIS_OFF:RowLen] = np.asarray(
        disamb_weight, dtype=np.float32).astype(BF).reshape(VOCAB, NUM_SENSE * D)
    # idx columns: 0..9 ctx, 10 word, 11..15 neg
    idx_all = np.concatenate(
        [context_ids, word_ids[:, None], neg_ids], axis=1).astype(np.int32)
    in_maps = []
    for c in range(N_CORES):
        sl = slice(c * PER_CORE, (c + 1) * PER_CORE)
        in_maps.append({
            "packed": packed,
            "idx": np.ascontiguousarray(idx_all[sl]),
        })
    return in_maps


def kernel(word_ids, context_ids, context_masks, neg_ids,
           emb_weight, global_emb_weight, disamb_weight):
    from concourse import bass_utils
    nc = _build_bass()
    in_maps = _prep_inputs(word_ids, context_ids, neg_ids,
                           emb_weight, global_emb_weight, disamb_weight)
    res = bass_utils.run_bass_kernel_spmd(nc, in_maps, core_ids=list(range(N_CORES)))
    total = 0.0
    for r in res.results:
        total += float(np.asarray(r["out"]).reshape(-1)[0])
    loss = -total / float(BATCH * CTX)
    return np.array(loss, dtype=np.float32)


# revision 4
# speedup vs baseline: 134.6091x; 1.0085x over previous
"""Multi-sense skip-gram (MSSG) loss kernel for Trainium2.

Data-parallel over batch across 8 cores; tables packed row-wise into one
[50000, 2100] bf16 table: row v = [global(300) | emb senses(900) | disamb
senses(900)].

Structure (per 128-element tile, 4 tiles/core; HW device time ~284us):
- 16 single-index indirect gathers with flat 2D SBUF dests. Multi-index
  gathers, 3D unit-dim dest views, and SBUF->SBUF accum_op DMAs all pass
  CoreSim but CRASH/corrupt real trn2 HW - do not reintroduce them.
- Dot products: big broadcast bf16 tensor_tensor multiplies (DVE 2x
  mode) + lopsided fold trees (150/74/38) + narrow tensor_reduce.
- Weighted sums: 30x tensor_scalar_mul + fold tree (scalar APs must be
  f32 - bf16 scalars are rejected by the API).
- pos/neg dot reductions on the Scalar (ACT) engine via per-segment
  Copy+accum_out; exp on ACT. Sigmoids are computed via the Exp table
  (exp/+1/reciprocal) and the loss Lns are deferred to after the tile
  loop so the loop keeps a single ACT table resident (saves ~1.3us per
  table reload).
"""

import numpy as np

NUM_SENSE = 3
EMB_DIM = 300
VOCAB = 50000
BATCH = 4096
CTX = 10
NEG = 5
N_CORES = 8
P = 128
PER_CORE = BATCH // N_CORES  # 512
TILES = PER_CORE // P        # 4
D = EMB_DIM
CS = CTX * NUM_SENSE         # 30
SN = NUM_SENSE * NEG         # 15
RowLen = D + 2 * NUM_SENSE * D  # 2100: [glob | emb | dis]
EMB_OFF = D                  # 300
DIS_OFF = D + NUM_SENSE * D  # 1200

_CACHE = {}


def _build_bass():
    key = "nc"
    if key in _CACHE:
        return _CACHE[key]

    import concourse.bass as bass
    import concourse.bacc as bacc
    import concourse.tile as tile
    from concourse import mybir

    F32 = mybir.dt.float32
    BF16 = mybir.dt.bfloat16
    I32 = mybir.dt.int32
    AX = mybir.AxisListType
    OP = mybir.AluOpType
    AF = mybir.ActivationFunctionType
    TINY = float(np.finfo(np.float32).tiny)

    nc = bacc.Bacc("TRN2", target_bir_lowering=False, debug=False)

    packed = nc.dram_tensor("packed", [VOCAB, RowLen], BF16, kind="ExternalInput")
    # idx columns: 0..9 ctx, 10 word, 11..15 neg
    idx = nc.dram_tensor("idx", [PER_CORE, 16], I32, kind="ExternalInput")
    out_d = nc.dram_tensor("out", [1, 1], F32, kind="ExternalOutput")

    def tt(out, a, b, op=OP.add):
        nc.vector.tensor_tensor(out=out, in0=a, in1=b, op=op)

    with tile.TileContext(nc) as tc:
        with (
            tc.tile_pool(name="gather", bufs=2) as gp,
            tc.tile_pool(name="tmpp", bufs=2) as tp,
            tc.tile_pool(name="small", bufs=2) as sp,
            tc.tile_pool(name="persist", bufs=1) as pp,
            tc.tile_pool(name="psum", bufs=1, space="PSUM") as psp,
        ):
            acc = pp.tile([P, 2 * TILES], F32)
            ones = pp.tile([P, 1], F32)
            WSAVE = pp.tile([P, TILES * CTX], F32)
            WNSAVE = pp.tile([P, TILES * NEG], F32)
            nc.vector.memset(ones[:], 1.0)

            for t in range(TILES):
                rows = slice(t * P, (t + 1) * P)
                ix = gp.tile([P, 16], I32)
                nc.sync.dma_start(out=ix[:], in_=idx[rows, :])

                PK = gp.tile([P, 11 * RowLen], BF16)   # 10 ctx + word, full rows
                NG = gp.tile([P, NEG * D], BF16)       # neg: glob part only
                PK3 = PK[:].rearrange("p (k x) -> p k x", x=RowLen)
                NG3 = NG[:].rearrange("p (n d) -> p n d", d=D)

                def gather1(dst, offs):
                    nc.gpsimd.indirect_dma_start(
                        out=dst, out_offset=None, in_=packed[:],
                        in_offset=bass.IndirectOffsetOnAxis(ap=offs, axis=0),
                    )

                for k in range(11):
                    gather1(PK[:, k * RowLen:(k + 1) * RowLen], ix[:, k:k + 1])
                for k in range(NEG):
                    gather1(NG[:, k * D:(k + 1) * D], ix[:, 11 + k:12 + k])

                CT3 = PK3[:, 0:CTX, 0:D]                               # [P,10,300]
                AS4 = PK3[:, 0:CTX, EMB_OFF:DIS_OFF].rearrange(
                    "p c (s d) -> p c s d", d=D)                       # [P,10,3,300]
                AD4 = PK3[:, 0:CTX, DIS_OFF:RowLen].rearrange(
                    "p c (s d) -> p c s d", d=D)
                SEN3 = PK3[:, CTX, EMB_OFF:DIS_OFF].rearrange(
                    "p (s d) -> p s d", d=D)                           # [P,3,300]
                DIS3 = PK3[:, CTX, DIS_OFF:RowLen].rearrange(
                    "p (s d) -> p s d", d=D)

                TMP = tp.tile([P, CS * D], BF16, tag="tmp_d")
                T3 = TMP[:].rearrange("p (k d) -> p k d", d=D)
                TMPQ = tp.tile([P, CS * D], BF16, tag="tmp_pn")
                Q3 = TMPQ[:].rearrange("p (k d) -> p k d", d=D)
                SCR = [sp.tile([P, D], BF16, tag=f"scr{i}", name=f"scr{i}")
                       for i in range(4)]

                def fold_reduce_30(src3, zout):
                    """src3 [P,30,300] bf16 products -> zout [P,30] f32 sums."""
                    tt(src3[:, :, 0:150], src3[:, :, 0:150], src3[:, :, 150:300])
                    tt(src3[:, :, 0:74], src3[:, :, 0:74], src3[:, :, 76:150])
                    tt(src3[:, :, 0:38], src3[:, :, 0:38], src3[:, :, 38:76])
                    nc.vector.tensor_reduce(
                        out=zout, in_=src3[:, :, 0:38], axis=AX.X, op=OP.add)

                def act_reduce(src3, zout, nseg):
                    """per-segment [P,300] Copy+accum on ACT -> zout[:, k]."""
                    for k in range(nseg):
                        nc.scalar.activation(
                            out=SCR[k % 4][:], in_=src3[:, k, :], func=AF.Copy,
                            accum_out=zout[:, k:k + 1])

                # ---- ctx1 = sum_c CT (mean deferred via exp scale) ----
                c1a = sp.tile([P, 5 * D], BF16)
                c1b = sp.tile([P, 2 * D], BF16)
                ctx1 = sp.tile([P, D], BF16)
                c1a3 = c1a[:].rearrange("p (c d) -> p c d", d=D)
                tt(c1a3, CT3[:, 0:5, :], CT3[:, 5:10, :])
                tt(c1b[:], c1a[:, 0:2 * D], c1a[:, 2 * D:4 * D])
                tt(c1b[:, 0:D], c1b[:, 0:D], c1b[:, D:2 * D])
                tt(ctx1[:], c1b[:, 0:D], c1a[:, 4 * D:5 * D])

                def disamb_step(ctx_vec, ctx_out, si):
                    # products + fold + reduce (DVE)
                    tt(T3.rearrange("p (c s) d -> p c s d", s=NUM_SENSE),
                       AD4,
                       ctx_vec[:].unsqueeze(1).unsqueeze(1)
                               .to_broadcast([P, CTX, NUM_SENSE, D]),
                       OP.mult)
                    z = sp.tile([P, CS], F32, tag=f"z{si}")
                    fold_reduce_30(T3, z[:])
                    # softmax over s (ACT exp, DVE small ops)
                    E = sp.tile([P, CS], F32, tag=f"E{si}")
                    nc.scalar.activation(out=E[:], in_=z[:], func=AF.Exp,
                                         scale=1.0 / CTX)
                    S = sp.tile([P, CTX], F32, tag=f"S{si}")
                    nc.vector.tensor_reduce(
                        out=S[:], in_=E[:].rearrange("p (c s) -> p c s",
                                                     s=NUM_SENSE),
                        axis=AX.X, op=OP.add)
                    R = sp.tile([P, CTX], F32, tag=f"R{si}")
                    nc.vector.reciprocal(R[:], S[:])
                    AL = sp.tile([P, CS], F32, tag=f"AL{si}")
                    tt(AL[:].rearrange("p (c s) -> p c s", s=NUM_SENSE),
                       E[:].rearrange("p (c s) -> p c s", s=NUM_SENSE),
                       R[:].unsqueeze(2).to_broadcast([P, CTX, NUM_SENSE]),
                       OP.mult)
                    # weighted sum: 30x tensor_scalar (4x mode) + fold tree
                    for k in range(CS):
                        nc.vector.tensor_scalar_mul(
                            out=T3[:, k, :],
                            in0=AS4[:, k // NUM_SENSE, k % NUM_SENSE, :],
                            scalar1=AL[:, k:k + 1])
                    tt(T3[:, 0:15, :], T3[:, 0:15, :], T3[:, 15:30, :])
                    tt(T3[:, 0:7, :], T3[:, 0:7, :], T3[:, 7:14, :])
                    tt(T3[:, 0:3, :], T3[:, 0:3, :], T3[:, 3:6, :])
                    tt(T3[:, 0:1, :], T3[:, 0:1, :], T3[:, 1:2, :])
                    tt(T3[:, 0:1, :], T3[:, 0:1, :], T3[:, 14:15, :])
                    tt(T3[:, 2:3, :], T3[:, 2:3, :], T3[:, 6:7, :])
                    tt(ctx_out[:].unsqueeze(1), T3[:, 0:1, :], T3[:, 2:3, :])

                ctx2 = sp.tile([P, D], BF16)
                ctx3 = sp.tile([P, D], BF16)
                disamb_step(ctx1, ctx2, 0)
                disamb_step(ctx2, ctx3, 1)

                # ---- alpha = softmax_s(DIS . ctx3 / CTX) ----
                tt(T3[:, 0:NUM_SENSE, :], DIS3,
                   ctx3[:].unsqueeze(1).to_broadcast([P, NUM_SENSE, D]),
                   OP.mult)
                av = sp.tile([P, NUM_SENSE], F32)
                a3 = T3[:, 0:NUM_SENSE, :]
                tt(a3[:, :, 0:150], a3[:, :, 0:150], a3[:, :, 150:300])
                tt(a3[:, :, 0:74], a3[:, :, 0:74], a3[:, :, 76:150])
                tt(a3[:, :, 0:38], a3[:, :, 0:38], a3[:, :, 38:76])
                nc.vector.tensor_reduce(
                    out=av[:], in_=a3[:, :, 0:38], axis=AX.X, op=OP.add)
                Ea = sp.tile([P, NUM_SENSE], F32)
                Sa = sp.tile([P, 1], F32)
                nc.scalar.activation(out=Ea[:], in_=av[:], func=AF.Exp,
                                     scale=1.0 / CTX, accum_out=Sa[:, 0:1])
                Ra = sp.tile([P, 1], F32)
                nc.vector.reciprocal(Ra[:], Sa[:])
                ALS = sp.tile([P, NUM_SENSE], F32)
                nc.vector.tensor_scalar_mul(out=ALS[:], in0=Ea[:],
                                            scalar1=Ra[:, 0:1])

                # ---- pos: sigmoid(SEN . CT) over (s,c) ----
                tt(Q3.rearrange("p (s c) d -> p s c d", c=CTX),
                   CT3.unsqueeze(1).to_broadcast([P, NUM_SENSE, CTX, D]),
                   SEN3.unsqueeze(2).to_broadcast([P, NUM_SENSE, CTX, D]),
                   OP.mult)
                PL = sp.tile([P, CS], F32)
                act_reduce(Q3, PL[:], CS)
                PP = sp.tile([P, CS], F32)
                nc.scalar.activation(out=PP[:], in_=PL[:], func=AF.Exp,
                                     scale=-1.0)
                nc.vector.tensor_scalar_add(out=PP[:], in0=PP[:], scalar1=1.0)
                nc.vector.reciprocal(PP[:], PP[:])

                W = sp.tile([P, CS], F32)
                for s in range(NUM_SENSE):
                    nc.vector.tensor_scalar_mul(
                        out=W[:, s * CTX:(s + 1) * CTX],
                        in0=PP[:, s * CTX:(s + 1) * CTX],
                        scalar1=ALS[:, s:s + 1])
                tt(W[:, 0:CTX], W[:, 0:CTX], W[:, CTX:2 * CTX])
                tt(W[:, 0:CTX], W[:, 0:CTX], W[:, 2 * CTX:3 * CTX])
                nc.vector.tensor_scalar_max(out=W[:, 0:CTX], in0=W[:, 0:CTX],
                                            scalar1=TINY)
                nc.vector.tensor_copy(out=WSAVE[:, t * CTX:(t + 1) * CTX],
                                      in_=W[:, 0:CTX])

                # ---- neg: sigmoid(SEN . NG) over (s,n) ----
                tt(Q3[:, 0:SN, :].rearrange("p (s n) d -> p s n d", n=NEG),
                   NG3.unsqueeze(1).to_broadcast([P, NUM_SENSE, NEG, D]),
                   SEN3.unsqueeze(2).to_broadcast([P, NUM_SENSE, NEG, D]),
                   OP.mult)
                NL = sp.tile([P, SN], F32)
                act_reduce(Q3, NL[:], SN)
                NP = sp.tile([P, SN], F32)
                nc.scalar.activation(out=NP[:], in_=NL[:], func=AF.Exp,
                                     scale=-1.0)
                nc.vector.tensor_scalar_add(out=NP[:], in0=NP[:], scalar1=1.0)
                nc.vector.reciprocal(NP[:], NP[:])

                Wn = sp.tile([P, SN], F32)
                for s in range(NUM_SENSE):
                    nc.vector.tensor_scalar_mul(
                        out=Wn[:, s * NEG:(s + 1) * NEG],
                        in0=NP[:, s * NEG:(s + 1) * NEG],
                        scalar1=ALS[:, s:s + 1])
                tt(Wn[:, 0:NEG], Wn[:, 0:NEG], Wn[:, NEG:2 * NEG])
                tt(Wn[:, 0:NEG], Wn[:, 0:NEG], Wn[:, 2 * NEG:3 * NEG])
                # 1 - x, clamp, ln
                nc.vector.tensor_scalar(
                    out=Wn[:, NEG:2 * NEG], in0=Wn[:, 0:NEG],
                    scalar1=-1.0, scalar2=1.0, op0=OP.mult, op1=OP.add)
                nc.vector.tensor_scalar_max(
                    out=Wn[:, NEG:2 * NEG], in0=Wn[:, NEG:2 * NEG], scalar1=TINY)
                nc.vector.tensor_copy(
                    out=WNSAVE[:, t * NEG:(t + 1) * NEG],
                    in_=Wn[:, NEG:2 * NEG])

            WL = pp.tile([P, TILES * CTX], F32)
            WLn = pp.tile([P, TILES * NEG], F32)
            for t in range(TILES):
                nc.scalar.activation(
                    out=WL[:, t * CTX:(t + 1) * CTX],
                    in_=WSAVE[:, t * CTX:(t + 1) * CTX], func=AF.Ln,
                    accum_out=acc[:, 2 * t:2 * t + 1])
            for t in range(TILES):
                nc.scalar.activation(
                    out=WLn[:, t * NEG:(t + 1) * NEG],
                    in_=WNSAVE[:, t * NEG:(t + 1) * NEG], func=AF.Ln,
                    accum_out=acc[:, 2 * t + 1:2 * t + 2])
            total = pp.tile([P, 1], F32)
            nc.vector.tensor_reduce(out=total[:], in_=acc[:], axis=AX.X, op=OP.add)
            ps = psp.tile([1, 1], F32)
            nc.tensor.matmul(out=ps[:], lhsT=total[:], rhs=ones[:],
                             start=True, stop=True)
            res = pp.tile([1, 1], F32)
            nc.scalar.copy(res[:], ps[:])
            nc.sync.dma_start(out=out_d[:, :], in_=res[:])

    nc.compile()
    _CACHE[key] = nc
    return nc


def _prep_inputs(word_ids, context_ids, neg_ids,
                 emb_weight, global_emb_weight, disamb_weight):
    import ml_dtypes
    BF = ml_dtypes.bfloat16
    word_ids = np.asarray(word_ids).astype(np.int32).reshape(BATCH)
    context_ids = np.asarray(context_ids).astype(np.int32).reshape(BATCH, CTX)
    neg_ids = np.asarray(neg_ids).astype(np.int32).reshape(BATCH, NEG)
    packed = np.empty((VOCAB, RowLen), dtype=BF)
    packed[:, 0:D] = np.asarray(
        global_emb_weight, dtype=np.float32).astype(BF).reshape(VOCAB, D)
    packed[:, EMB_OFF:DIS_OFF] = np.asarray(
        emb_weight, dtype=np.float32).astype(BF).reshape(VOCAB, NUM_SENSE * D)
    packed[:, DIS_OFF:RowLen] = np.asarray(
        disamb_weight, dtype=np.float32).astype(BF).reshape(VOCAB, NUM_SENSE * D)
    # idx columns: 0..9 ctx, 10 word, 11..15 neg
    idx_all = np.concatenate(
        [context_ids, word_ids[:, None], neg_ids], axis=1).astype(np.int32)
    in_maps = []
    for c in range(N_CORES):
        sl = slice(c * PER_CORE, (c + 1) * PER_CORE)
        in_maps.append({
            "packed": packed,
            "idx": np.ascontiguousarray(idx_all[sl]),
        })
    return in_maps


def kernel(word_ids, context_ids, context_masks, neg_ids,
           emb_weight, global_emb_weight, disamb_weight):
    from concourse import bass_utils
    nc = _build_bass()
    in_maps = _prep_inputs(word_ids, context_ids, neg_ids,
                           emb_weight, global_emb_weight, disamb_weight)
    res = bass_utils.run_bass_kernel_spmd(nc, in_maps, core_ids=list(range(N_CORES)))
    total = 0.0
    for r in res.results:
        total += float(np.asarray(r["out"]).reshape(-1)[0])
    loss = -total / float(BATCH * CTX)
    return np.array(loss, dtype=np.float32)
